# revision 21
# baseline (speedup 1.0000x reference)
"""Fused distributed Trainium2 kernel for nn_Attention_73564199846195.

The entire network runs on the 8 NeuronCores in ONE SPMD kernel launch:
  base = w_qkv @ x            (1x1 conv, PE, bf16)
  dwout = concat(dw3,dw5,dw7) (depthwise convs: PE diag-matmuls + DVE/GpSimd
                               MAC chains, bf16 with fp32 accum on GP)
  qkv  = w_pconv @ dwout      (dominant matmul, PE, bf16)
  gram = q @ k^T              (per-pixel-shard partial, PE, accumulated in
                               PSUM across the whole sweep)
  l2-norm stats + gram        -> tiny grouped AllReduce (4 cores per batch)
  top-k masked softmax x3     (fused: thresholds from max8 extraction,
                               single combined weight matrix W)
  out  = W @ v ; y = w_proj @ out

Sharding: pixels (batch x 32 image rows per core, 3-row halo shipped from
host, zero-padded W so depthwise shifts are pure AP offsets). Weights are
sharded 1/8 per core on the wire and AllGathered on-device (the axon
host<->device link at ~17 MB/s is the real bottleneck; bytes shipped are
minimized: fp16 everywhere, ~65 MB total vs ~730 MB for the baseline).
"""

from contextlib import ExitStack

import numpy as np

import concourse.bacc as bacc
import concourse.mybir as mybir
from concourse import tile
from concourse.bass_utils import run_bass_kernel_spmd

F16 = mybir.dt.float16
F32 = mybir.dt.float32
F32R = mybir.dt.float32r
AX = mybir.AxisListType
OP = mybir.AluOpType
ACT = mybir.ActivationFunctionType

N_CORES = 8
C = 384            # model dim
HEADS = 8
CH = C // HEADS    # 48 channels per head
H = 128
W = 128
B = 2
ROWS = 32          # image rows per core
HALO = 3
WP = W + 6         # zero-padded row width (134)
EXTR = ROWS + 2 * HALO          # 38 ext rows
XCOLS = EXTR * WP               # 5092
NLOC = ROWS * W                 # 4096 local pixels
CHUNK = 512                     # pixels per chunk
NCHUNKS = NLOC // CHUNK
CROWS = CHUNK // W              # image rows per chunk
KTS = CHUNK // 128              # 128-px transpose blocks per chunk
QTRS = 4                        # base production granularity: 8 rows
QROWS = ROWS // QTRS            # 8 valid rows per quarter
QEXT = (QROWS + 2 * HALO) * WP  # 1876 ext px per quarter

M3 = C // 128      # 3   (128-row tiles in C)
M9 = 3 * C // 128  # 9   (tiles in 3C=1152)
K27 = 9 * C // 128 # 27  (tiles in 9C=3456)

KSIZES = (3, 5, 7)
NTAPS = tuple(k * k for k in KSIZES)    # 9, 25, 49
DWW_OFF = (0, 81, 81 + 225)             # dww column offset per j (x9 blocks)
DWW_COLS = 81 + 225 + 441               # 747

# engine assignment per (j, bb): "pe" | "dve" | "gp"   (tuning knob)
DW_ASSIGN = {}
for _bb in range(9):
    DW_ASSIGN[(0, _bb)] = "pe"
    DW_ASSIGN[(1, _bb)] = "pe"
    DW_ASSIGN[(2, _bb)] = ("pe" if _bb < 3 else
                           ("dve" if _bb < 7 else "gp"))

GRAM_N = HEADS * CH * CH        # 18432
BN_TOT = GRAM_N + 2 * C         # 19200 f32 allreduce payload

WQ_ROWS, WQ_COLS = C, 3 * C            # 384 x 1152
WPC_ROWS, WPC_COLS = 9 * C, 3 * C      # 3456 x 1152
WPR_ROWS, WPR_COLS = CH, HEADS * C     # 48 x 3072

_cached = {}
last_exec_time_ns = None
TRACE = False


def _pair_of_head(h):
    return h // 2, 64 * (h % 2)


def _build(attn_scales):
    s0, s1, s2 = [float(v) for v in attn_scales]
    nc = bacc.Bacc("TRN2", target_bir_lowering=False, debug=False,
                   num_devices=N_CORES)

    # ---- external params (per core) ----
    x_ext = nc.declare_dram_parameter("x", [C, XCOLS], F16, isOutput=False)
    wq_sh = nc.declare_dram_parameter("wq", [WQ_ROWS // 8, WQ_COLS], F16,
                                      isOutput=False)
    wp_sh = nc.declare_dram_parameter("wp", [WPC_ROWS // 8, WPC_COLS], F16,
                                      isOutput=False)
    wr_sh = nc.declare_dram_parameter("wr", [WPR_ROWS // 8, WPR_COLS], F16,
                                      isOutput=False)
    dww = nc.declare_dram_parameter("dww", [128, DWW_COLS], F32,
                                    isOutput=False)
    colgrid = nc.declare_dram_parameter("colgrid", [128, 128], F32,
                                        isOutput=False)
    rowval = nc.declare_dram_parameter("rowval", [128, 1], F32,
                                       isOutput=False)
    hsel = nc.declare_dram_parameter("hsel", [8, 512], F32, isOutput=False)
    temp4 = nc.declare_dram_parameter("temp4", [128, 4], F32, isOutput=False)
    y_out = nc.declare_dram_parameter("y", [C, NLOC], F16, isOutput=True)

    # ---- internal DRAM ----
    wq_in = nc.dram_tensor("wq_in", [WQ_ROWS // 8, WQ_COLS], F16)
    wp_in = nc.dram_tensor("wp_in", [WPC_ROWS // 8, WPC_COLS], F16)
    wr_in = nc.dram_tensor("wr_in", [WPR_ROWS // 8, WPR_COLS], F16)
    wq_d = nc.dram_tensor("wq_d", [WQ_ROWS, WQ_COLS], F16, addr_space="Shared")
    wp_d = nc.dram_tensor("wp_d", [WPC_ROWS, WPC_COLS], F16, addr_space="Shared")
    wr_d = nc.dram_tensor("wr_d", [WPR_ROWS, WPR_COLS], F16, addr_space="Shared")
    v_d = nc.dram_tensor("v_d", [C, NLOC], F16)
    bn_in = nc.dram_tensor("bn_in", [BN_TOT], F32)
    bn_out = nc.dram_tensor("bn_out", [BN_TOT], F32)

    groups8 = [list(range(N_CORES))]
    groups4 = [[0, 1, 2, 3], [4, 5, 6, 7]]

    with tile.TileContext(nc) as tc, ExitStack() as est:
        wpool = est.enter_context(tc.tile_pool(name="weights", bufs=1))
        cpool = est.enter_context(tc.tile_pool(name="consts", bufs=1))

        # ---- weight AllGather ----
        nc.sync.dma_start(wq_in[:], wq_sh[:])
        nc.sync.dma_start(wp_in[:], wp_sh[:])
        nc.sync.dma_start(wr_in[:], wr_sh[:])
        nc.gpsimd.collective_compute(
            "AllGather", OP.bypass, replica_groups=groups8,
            ins=[wq_in[:].opt()], outs=[wq_d[:].opt()])
        nc.gpsimd.collective_compute(
            "AllGather", OP.bypass, replica_groups=groups8,
            ins=[wp_in[:].opt()], outs=[wp_d[:].opt()])
        nc.gpsimd.collective_compute(
            "AllGather", OP.bypass, replica_groups=groups8,
            ins=[wr_in[:].opt()], outs=[wr_d[:].opt()])

        wq_sb = []
        for k in range(M3):
            t = wpool.tile([128, WQ_COLS], F16, tag=f"wq{k}")
            nc.sync.dma_start(t[:], wq_d[k * 128:(k + 1) * 128, :])
            wq_sb.append(t)
        wp_sb = []
        for k in range(K27):
            t = wpool.tile([128, WPC_COLS], F16, tag=f"wp{k}")
            nc.sync.dma_start(t[:], wp_d[k * 128:(k + 1) * 128, :])
            wp_sb.append(t)
        wr_sb = wpool.tile([WPR_ROWS, WPR_COLS], F16, tag="wr")
        nc.sync.dma_start(wr_sb[:], wr_d[:])

        dww_sb = cpool.tile([128, DWW_COLS], F32, tag="dww")
        nc.sync.dma_start(dww_sb[:], dww[:])
        colg_sb = cpool.tile([128, 128], F32, tag="colg")
        nc.sync.dma_start(colg_sb[:], colgrid[:])
        rowv_sb = cpool.tile([128, 1], F32, tag="rowv")
        nc.sync.dma_start(rowv_sb[:], rowval[:])
        hsel_sb = cpool.tile([8, 512], F32, tag="hsel")
        nc.sync.dma_start(hsel_sb[:], hsel[:])
        temp_sb = cpool.tile([128, 4], F32, tag="temp")
        nc.sync.dma_start(temp_sb[:], temp4[:])
        ident_f = cpool.tile([128, 128], F32, tag="idf")
        nc.vector.tensor_scalar(ident_f[:], colg_sb[:], rowv_sb[:, 0:1],
                                None, op0=OP.is_equal)
        ident_b = cpool.tile([128, 128], F16, tag="idb")
        nc.vector.tensor_copy(ident_b[:], ident_f[:])
        hselb = cpool.tile([8, 512], F16, tag="hselb")
        nc.vector.tensor_copy(hselb[:], hsel_sb[:])

        # ============ mid phase: sweep + stats + allreduce ============
        with ExitStack() as mid:
            gram_pool = mid.enter_context(
                tc.tile_pool(name="gram", bufs=1, space="PSUM"))
            stat_pool = mid.enter_context(tc.tile_pool(name="stats", bufs=1))
            gram_ps = [gram_pool.tile([128, C], F32, tag=f"g{m}", name=f"g{m}")
                       for m in range(M3)]
            qstats = [stat_pool.tile([128, NCHUNKS], F32, tag=f"qs{m}", name=f"qs{m}")
                      for m in range(M3)]
            kstats = [stat_pool.tile([128, NCHUNKS], F32, tag=f"ks{m}", name=f"ks{m}")
                      for m in range(M3)]

            with ExitStack() as sweep:
                bpool = sweep.enter_context(tc.tile_pool(name="base", bufs=1))
                xpool = sweep.enter_context(tc.tile_pool(name="xin", bufs=2))
                dopool = sweep.enter_context(
                    tc.tile_pool(name="dwout", bufs=28))
                dgpool = sweep.enter_context(tc.tile_pool(name="diag",
                                                          bufs=2))
                pspool = sweep.enter_context(
                    tc.tile_pool(name="ps", bufs=3, space="PSUM"))
                tppool = sweep.enter_context(
                    tc.tile_pool(name="tp", bufs=2, space="PSUM"))
                qkpool = sweep.enter_context(tc.tile_pool(name="qk", bufs=2))
                qtpool = sweep.enter_context(tc.tile_pool(name="qkT",
                                                          bufs=4))
                vpool = sweep.enter_context(tc.tile_pool(name="vsb", bufs=2))
                acpool = sweep.enter_context(tc.tile_pool(name="acc",
                                                          bufs=1))
                sqpool = sweep.enter_context(tc.tile_pool(name="sq", bufs=2))

                gchunk = 0
                for q in range(QTRS):
                    # ---- base = wqkv @ x over ext quarter ----
                    base_sb = [bpool.tile([128, QEXT], F16, tag=f"b{m}", name=f"b{m}")
                               for m in range(M9)]
                    qoff = (q * QROWS) * WP
                    for c0 in range(0, QEXT, CHUNK):
                        cw = min(CHUNK, QEXT - c0)
                        xt = xpool.tile([128, M3 * CHUNK], F16, tag="xt")
                        for kb in range(M3):
                            nc.sync.dma_start(
                                xt[:, kb * CHUNK:kb * CHUNK + cw],
                                x_ext[kb * 128:(kb + 1) * 128,
                                      qoff + c0:qoff + c0 + cw])
                        for m in range(M9):
                            ps = pspool.tile([128, CHUNK], F32, tag="ps")
                            for kb in range(M3):
                                nc.tensor.matmul(
                                    ps[:, :cw],
                                    wq_sb[kb][:, m * 128:(m + 1) * 128],
                                    xt[:, kb * CHUNK:kb * CHUNK + cw],
                                    start=(kb == 0), stop=(kb == M3 - 1))
                            nc.scalar.copy(base_sb[m][:, c0:c0 + cw],
                                           ps[:, :cw])

                    base3 = [b[:].rearrange("p (r w) -> p r w", w=WP)
                             for b in base_sb]

                    # ---- dwout + pconv ----
                    for ck in range(QROWS // CROWS):
                        r0 = ck * CROWS + HALO
                        dwt = {}
                        for j in range(3):
                            for bb in range(M9):
                                dwt[(j, bb)] = dopool.tile(
                                    [128, CHUNK], F16, tag="dw",
                                    name="dwt")
                        # PE blocks: diag matmuls
                        for bb in range(M9):
                            pe_js = [j for j in range(3)
                                     if DW_ASSIGN[(j, bb)] == "pe"]
                            for j in pe_js:
                                kk = KSIZES[j]
                                pad = kk // 2
                                nt = NTAPS[j]
                                dg = dgpool.tile([128, nt * 128], F16,
                                                 tag=f"dg{j}", bufs=1,
                                                 name=f"dg{j}")
                                for t in range(nt):
                                    col = DWW_OFF[j] + bb * nt + t
                                    nc.vector.tensor_scalar(
                                        dg[:, t * 128:(t + 1) * 128],
                                        colg_sb[:],
                                        rowv_sb[:, 0:1],
                                        dww_sb[:, col:col + 1],
                                        op0=OP.is_equal, op1=OP.mult)
                                ps = pspool.tile([128, CHUNK], F32, tag="ps")
                                for t in range(nt):
                                    dy = t // kk - pad
                                    dx = t % kk - pad
                                    rhs = base3[bb][
                                        :, r0 + dy:r0 + dy + CROWS,
                                        HALO + dx:HALO + dx + W]
                                    nc.tensor.matmul(
                                        ps[:],
                                        dg[:, t * 128:(t + 1) * 128],
                                        rhs,
                                        start=(t == 0), stop=(t == nt - 1))
                                nc.scalar.copy(dwt[(j, bb)][:], ps[:])
                        # DVE / GP blocks
                        for j in range(3):
                            kk = KSIZES[j]
                            pad = kk // 2
                            nt = NTAPS[j]
                            for bb in range(M9):
                                eng = DW_ASSIGN[(j, bb)]
                                if eng == "pe":
                                    continue
                                out3 = dwt[(j, bb)][:].rearrange(
                                    "p (r w) -> p r w", w=W)
                                if eng == "gp":
                                    acc = acpool.tile([128, CHUNK], F32,
                                                      tag="gacc", bufs=2)
                                    acc3 = acc[:].rearrange(
                                        "p (r w) -> p r w", w=W)
                                    for t in range(nt):
                                        dy = t // kk - pad
                                        dx = t % kk - pad
                                        col = DWW_OFF[j] + bb * nt + t
                                        src = base3[bb][
                                            :, r0 + dy:r0 + dy + CROWS,
                                            HALO + dx:HALO + dx + W]
                                        if t == 0:
                                            nc.gpsimd.tensor_scalar(
                                                acc3, src,
                                                dww_sb[:, col:col + 1], None,
                                                op0=OP.mult)
                                            continue
                                        prod = acpool.tile(
                                            [128, CHUNK], F16,
                                            tag="gprod", bufs=2)
                                        prod3 = prod[:].rearrange(
                                            "p (r w) -> p r w", w=W)
                                        nc.gpsimd.tensor_scalar(
                                            prod3, src,
                                            dww_sb[:, col:col + 1], None,
                                            op0=OP.mult)
                                        dst = (dwt[(j, bb)][:]
                                               if t == nt - 1 else acc[:])
                                        nc.gpsimd.tensor_tensor(
                                            dst, acc[:], prod[:], op=OP.add)
                                else:  # dve: fused stt chain, f32 accum
                                    acc = acpool.tile([128, CHUNK], F32,
                                                      tag="dacc", bufs=2)
                                    acc3 = acc[:].rearrange(
                                        "p (r w) -> p r w", w=W)
                                    for t in range(nt):
                                        dy = t // kk - pad
                                        dx = t % kk - pad
                                        col = DWW_OFF[j] + bb * nt + t
                                        src = base3[bb][
                                            :, r0 + dy:r0 + dy + CROWS,
                                            HALO + dx:HALO + dx + W]
                                        if t == 0:
                                            nc.vector.tensor_scalar(
                                                acc3, src,
                                                dww_sb[:, col:col + 1], None,
                                                op0=OP.mult)
                                        else:
                                            dst = (dwt[(j, bb)][:]
                                                   .rearrange(
                                                       "p (r w) -> p r w",
                                                       w=W)
                                                   if t == nt - 1 else acc3)
                                            nc.vector.scalar_tensor_tensor(
                                                dst, src,
                                                dww_sb[:, col:col + 1],
                                                acc3,
                                                op0=OP.mult, op1=OP.add)

                        # ---- pconv for this chunk ----
                        q_sb = qkpool.tile([128, M3 * CHUNK], F16,
                                           tag="qsb")
                        k_sb = qkpool.tile([128, M3 * CHUNK], F16,
                                           tag="ksb")
                        v_sb = vpool.tile([128, M3 * CHUNK], F16, tag="vsb")
                        for m in range(M9):
                            ps = pspool.tile([128, CHUNK], F32, tag="ps")
                            for kb in range(K27):
                                j, bb = kb // 9, kb % 9
                                nc.tensor.matmul(
                                    ps[:],
                                    wp_sb[kb][:, m * 128:(m + 1) * 128],
                                    dwt[(j, bb)][:],
                                    start=(kb == 0), stop=(kb == K27 - 1))
                            grp = m // M3
                            sub = m % M3
                            dst = (q_sb, k_sb, v_sb)[grp]
                            nc.scalar.copy(
                                dst[:, sub * CHUNK:(sub + 1) * CHUNK],
                                ps[:])

                        # v spill to DRAM
                        for sub in range(M3):
                            nc.sync.dma_start(
                                v_d[sub * 128:(sub + 1) * 128,
                                    gchunk * CHUNK:(gchunk + 1) * CHUNK],
                                v_sb[:, sub * CHUNK:(sub + 1) * CHUNK])

                        # sumsq partials (ACT engine, accum_out)
                        for m in range(M3):
                            sq = sqpool.tile([128, CHUNK], F16, tag="sq")
                            nc.scalar.activation(
                                sq[:], q_sb[:, m * CHUNK:(m + 1) * CHUNK],
                                ACT.Square,
                                accum_out=qstats[m][:, gchunk:gchunk + 1])
                            sq2 = sqpool.tile([128, CHUNK], F16, tag="sq")
                            nc.scalar.activation(
                                sq2[:], k_sb[:, m * CHUNK:(m + 1) * CHUNK],
                                ACT.Square,
                                accum_out=kstats[m][:, gchunk:gchunk + 1])

                        # transposes -> qT, kT  [px, C] bf16
                        qT = [qtpool.tile([128, C], F16, tag="qT", name="qTt")
                              for _ in range(KTS)]
                        kT = [qtpool.tile([128, C], F16, tag="kT", name="kTt")
                              for _ in range(KTS)]
                        for kt in range(KTS):
                            for m in range(M3):
                                tp = tppool.tile([128, 128], F16, tag="tp")
                                nc.tensor.transpose(
                                    tp[:],
                                    q_sb[:, m * CHUNK + kt * 128:
                                         m * CHUNK + (kt + 1) * 128],
                                    ident_b[:])
                                nc.scalar.copy(
                                    qT[kt][:, m * 128:(m + 1) * 128], tp[:])
                                tp2 = tppool.tile([128, 128], F16,
                                                  tag="tp")
                                nc.tensor.transpose(
                                    tp2[:],
                                    k_sb[:, m * CHUNK + kt * 128:
                                         m * CHUNK + (kt + 1) * 128],
                                    ident_b[:])
                                nc.scalar.copy(
                                    kT[kt][:, m * 128:(m + 1) * 128],
                                    tp2[:])
                        for kt in range(KTS):
                            for m in range(M3):
                                nc.tensor.matmul(
                                    gram_ps[m][:],
                                    qT[kt][:, m * 128:(m + 1) * 128],
                                    kT[kt][:],
                                    start=(gchunk == 0 and kt == 0),
                                    stop=(gchunk == NCHUNKS - 1
                                          and kt == KTS - 1),
                                    skip_group_check=True)
                        gchunk += 1

            # ---- stats finish + bounce out + allreduce ----
            with ExitStack() as bst:
                bpool2 = bst.enter_context(tc.tile_pool(name="bnc", bufs=1))
                gsb = [bpool2.tile([128, C], F32, tag=f"gsb{m}", name=f"gsb{m}")
                       for m in range(M3)]
                for m in range(M3):
                    nc.scalar.copy(gsb[m][:], gram_ps[m][:])
                qsq = [bpool2.tile([128, 1], F32, tag=f"qq{m}", name=f"qqs{m}")
                       for m in range(M3)]
                ksq = [bpool2.tile([128, 1], F32, tag=f"kq{m}", name=f"kqs{m}")
                       for m in range(M3)]
                for m in range(M3):
                    nc.vector.tensor_reduce(qsq[m][:], qstats[m][:],
                                            axis=AX.X, op=OP.add)
                    nc.vector.tensor_reduce(ksq[m][:], kstats[m][:],
                                            axis=AX.X, op=OP.add)
                for h in range(HEADS):
                    rlo = h * CH
                    rhi = rlo + CH
                    dst0 = h * CH * CH
                    while rlo < rhi:
                        mt = rlo // 128
                        r_in = rlo % 128
                        nrow = min(rhi - rlo, 128 - r_in)
                        nc.sync.dma_start(
                            bn_in[dst0:dst0 + nrow * CH],
                            gsb[mt][r_in:r_in + nrow, h * CH:(h + 1) * CH])
                        dst0 += nrow * CH
                        rlo += nrow
                for m in range(M3):
                    nc.sync.dma_start(
                        bn_in[GRAM_N + m * 128:GRAM_N + (m + 1) * 128],
                        qsq[m][:])
                    nc.sync.dma_start(
                        bn_in[GRAM_N + C + m * 128:
                              GRAM_N + C + (m + 1) * 128],
                        ksq[m][:])
                nc.gpsimd.collective_compute(
                    "AllReduce", OP.add, replica_groups=groups4,
                    ins=[bn_in[:].opt()], outs=[bn_out[:].opt()])

        # ============ tail: softmax + output ============
        with ExitStack() as tail:
            spool = tail.enter_context(tc.tile_pool(name="smax", bufs=1))
            wkpool = tail.enter_context(tc.tile_pool(name="wk", bufs=2))
            smx = ExitStack()
            rkps = smx.enter_context(
                tc.tile_pool(name="rkps", bufs=2, space="PSUM"))

            rk_sb = spool.tile([8, CH], F32, tag="rk")
            nc.sync.dma_start(rk_sb[:], bn_out[GRAM_N + C:GRAM_N + 2 * C])
            nc.scalar.sqrt(rk_sb[:], rk_sb[:])
            nc.vector.tensor_scalar_max(rk_sb[:], rk_sb[:], 1e-12)
            nc.vector.reciprocal(rk_sb[:], rk_sb[:])

            WT_sb = []
            for t in range(4):
                attn = wkpool.tile([128, CH], F32, tag="attn")
                nc.vector.memset(attn[:], 0.0)
                for o, hh in ((0, 2 * t), (64, 2 * t + 1)):
                    nc.sync.dma_start(
                        attn[o:o + CH, :],
                        bn_out[hh * CH * CH:(hh + 1) * CH * CH])
                qq = wkpool.tile([128, 1], F32, tag="qq")
                nc.vector.memset(qq[:], 1.0)
                for o, hh in ((0, 2 * t), (64, 2 * t + 1)):
                    nc.sync.dma_start(
                        qq[o:o + CH, :],
                        bn_out[GRAM_N + hh * CH:GRAM_N + (hh + 1) * CH])
                nc.scalar.sqrt(qq[:], qq[:])
                nc.vector.tensor_scalar_max(qq[:], qq[:], 1e-12)
                nc.vector.reciprocal(qq[:], qq[:])
                nc.vector.tensor_tensor(qq[:], qq[:], temp_sb[:, t:t + 1],
                                        op=OP.mult)
                rkb = rkps.tile([128, CH], F32, tag="rkb")
                rkb16 = wkpool.tile([8, CH], F16, tag="rkb16")
                nc.vector.tensor_copy(rkb16[:], rk_sb[:])
                nc.tensor.matmul(rkb[:],
                                 hselb[:, t * 128:(t + 1) * 128],
                                 rkb16[:])
                nc.vector.tensor_tensor(attn[:], attn[:], rkb[:],
                                        op=OP.mult)
                nc.vector.tensor_scalar(attn[:], attn[:], qq[:, 0:1],
                                        None, op0=OP.mult)
                # top-40 extraction
                srt = wkpool.tile([128, 40], F32, tag="srt")
                wrk = wkpool.tile([128, CH], F32, tag="wrk")
                nc.vector.tensor_copy(wrk[:], attn[:])
                for it in range(5):
                    nc.vector.max(srt[:, it * 8:(it + 1) * 8], wrk[:])
                    nc.vector.match_replace(
                        wrk[:], srt[:, it * 8:(it + 1) * 8], wrk[:], -1e30)
                nrm = wkpool.tile([128, 1], F32, tag="nrm")
                nc.vector.tensor_scalar_mul(nrm[:], srt[:, 0:1], -1.0)
                es = wkpool.tile([128, 40], F32, tag="es")
                nc.scalar.activation(es[:], srt[:], ACT.Exp,
                                     bias=nrm[:, 0:1])
                dd = wkpool.tile([128, 3], F32, tag="dd")
                nc.vector.tensor_reduce(dd[:, 0:1], es[:, 0:24],
                                        axis=AX.X, op=OP.add)
                nc.vector.tensor_reduce(dd[:, 1:2], es[:, 24:32],
                                        axis=AX.X, op=OP.add)
                nc.vector.tensor_reduce(dd[:, 2:3], es[:, 32:36],
                                        axis=AX.X, op=OP.add)
                nc.vector.tensor_tensor(dd[:, 1:2], dd[:, 1:2], dd[:, 0:1],
                                        op=OP.add)
                nc.vector.tensor_tensor(dd[:, 2:3], dd[:, 2:3], dd[:, 1:2],
                                        op=OP.add)
                rr = wkpool.tile([128, 3], F32, tag="rr")
                nc.vector.reciprocal(rr[:], dd[:])
                for i, sc in enumerate((s0, s1, s2)):
                    nc.scalar.mul(rr[:, i:i + 1], rr[:, i:i + 1], float(sc))
                Wt = wkpool.tile([128, CH], F16, tag="Wt")
                tmp = wkpool.tile([128, CH], F16, tag="tmpC")
                for i, kidx in enumerate((24, 32, 36)):
                    dst = Wt if i == 0 else tmp
                    nc.vector.tensor_scalar(
                        dst[:], attn[:],
                        srt[:, kidx - 1:kidx],
                        rr[:, i:i + 1],
                        op0=OP.is_ge, op1=OP.mult)
                    if i > 0:
                        nc.vector.tensor_tensor(Wt[:], Wt[:], tmp[:],
                                                op=OP.add)
                EE = wkpool.tile([128, CH], F32, tag="EE")
                nc.scalar.activation(EE[:], attn[:], ACT.Exp,
                                     bias=nrm[:, 0:1])
                nc.vector.tensor_tensor(Wt[:], Wt[:], EE[:], op=OP.mult)
                wt_t = spool.tile([128, 128], F16, tag=f"WT{t}")
                for off in (0, 64):
                    tpw = rkps.tile([128, 128], F16, tag="tpw")
                    nc.tensor.transpose(
                        tpw[off:off + CH, :],
                        Wt[:],
                        ident_b[:])
                    nc.scalar.copy(wt_t[off:off + CH, :],
                                   tpw[off:off + CH, :])
                WT_sb.append(wt_t)

            smx.close()

            # ---- out = W @ v ; y = wproj @ out ----
            with ExitStack() as ph3:
                vtp = ph3.enter_context(tc.tile_pool(name="vt", bufs=8))
                ohp = ph3.enter_context(tc.tile_pool(name="oh", bufs=3))
                yp = ph3.enter_context(tc.tile_pool(name="ysb", bufs=2))
                ops_ = ph3.enter_context(
                    tc.tile_pool(name="ops", bufs=2, space="PSUM"))
                yps = ph3.enter_context(
                    tc.tile_pool(name="yps", bufs=2, space="PSUM"))
                for g in range(NCHUNKS):
                    vt = [vtp.tile([128, CHUNK], F16, tag="vt", name="vtt")
                          for _ in range(4)]
                    for h in range(HEADS):
                        pr, off = _pair_of_head(h)
                        nc.sync.dma_start(
                            vt[pr][off:off + CH, :],
                            v_d[h * CH:(h + 1) * CH,
                                g * CHUNK:(g + 1) * CHUNK])
                    y_ps = [yps.tile([128, CHUNK], F32, tag=f"yps{m}", name=f"ypst{m}")
                            for m in range(M3)]
                    for h in range(HEADS):
                        pr, off = _pair_of_head(h)
                        po = ops_.tile([CH, CHUNK], F32, tag="po")
                        nc.tensor.matmul(
                            po[:],
                            WT_sb[pr][off:off + CH, off:off + CH],
                            vt[pr][off:off + CH, :])
                        oh = ohp.tile([CH, CHUNK], F16, tag="oh")
                        nc.scalar.copy(oh[:], po[:])
                        for m in range(M3):
                            nc.tensor.matmul(
                                y_ps[m][:],
                                wr_sb[:, h * C + m * 128:
                                      h * C + (m + 1) * 128],
                                oh[:],
                                start=(h == 0), stop=(h == HEADS - 1))
                    for m in range(M3):
                        ysb = yp.tile([128, CHUNK], F16, tag="ysb")
                        nc.scalar.copy(ysb[:], y_ps[m][:])
                        nc.sync.dma_start(
                            y_out[m * 128:(m + 1) * 128,
                                  g * CHUNK:(g + 1) * CHUNK],
                            ysb[:])
    nc.compile()
    return nc


def _get_nc(attn_scales):
    key = tuple(float(v) for v in attn_scales)
    if key not in _cached:
        _cached[key] = _build(attn_scales)
    return _cached[key]


def _prep_host(x, w_qkv, w_dw3, w_dw5, w_dw7, w_pconv, w_proj, temperature):
    bf = np.float16
    xs = []
    xf = np.asarray(x, np.float32)
    for core in range(N_CORES):
        b, s = core // 4, core % 4
        r0 = s * ROWS - HALO
        xe = np.zeros((C, EXTR, WP), np.float32)
        lo, hi = max(r0, 0), min(r0 + EXTR, H)
        xe[:, lo - r0:hi - r0, HALO:HALO + W] = xf[b, :, lo:hi, :]
        xs.append(np.ascontiguousarray(xe.reshape(C, XCOLS)).astype(bf))
    wqT = np.ascontiguousarray(np.asarray(w_qkv, np.float32).T).astype(bf)
    wpT = np.ascontiguousarray(np.asarray(w_pconv, np.float32).T).astype(bf)
    wproj = np.asarray(w_proj, np.float32)
    wpr = np.zeros((CH, HEADS * C), np.float32)
    for h in range(HEADS):
        wpr[:, h * C:(h + 1) * C] = wproj[:, h * CH:(h + 1) * CH].T
    wpr = wpr.astype(bf)

    dwwp = np.zeros((128, DWW_COLS), np.float32)
    for j, wdw in enumerate((w_dw3, w_dw5, w_dw7)):
        kk = KSIZES[j]
        wv = np.asarray(wdw, np.float32).reshape(3 * C, kk * kk)
        for bb in range(M9):
            dwwp[:, DWW_OFF[j] + bb * NTAPS[j]:
                 DWW_OFF[j] + (bb + 1) * NTAPS[j]] = \
                wv[bb * 128:(bb + 1) * 128, :]

    colgrid = np.tile(np.arange(128, dtype=np.float32), (128, 1))
    colgrid = np.ascontiguousarray(colgrid)
    rowval = np.arange(128, dtype=np.float32).reshape(128, 1).copy()
    hselp = np.zeros((8, 512), np.float32)
    temp4p = np.zeros((128, 4), np.float32)
    tempv = np.asarray(temperature, np.float32).reshape(HEADS)
    for t in range(4):
        for o, hh in ((0, 2 * t), (64, 2 * t + 1)):
            hselp[hh, t * 128 + o:t * 128 + o + CH] = 1.0
            temp4p[o:o + CH, t] = tempv[hh]
    consts = dict(dww=dwwp, colgrid=colgrid, rowval=rowval, hsel=hselp,
                  temp4=temp4p)
    return xs, wqT, wpT, wpr, consts


def kernel(x, w_qkv, w_dw3, w_dw5, w_dw7, w_pconv, w_proj, temperature,
           attn_scales):
    global last_exec_time_ns
    nc = _get_nc(np.asarray(attn_scales, np.float32))
    xs, wqT, wpT, wpr, consts = _prep_host(
        x, w_qkv, w_dw3, w_dw5, w_dw7, w_pconv, w_proj, temperature)
    in_maps = []
    for core in range(N_CORES):
        sq, sp, sr = WQ_ROWS // 8, WPC_ROWS // 8, WPR_ROWS // 8
        m = {"x": xs[core],
             "wq": np.ascontiguousarray(wqT[core * sq:(core + 1) * sq]),
             "wp": np.ascontiguousarray(wpT[core * sp:(core + 1) * sp]),
             "wr": np.ascontiguousarray(wpr[core * sr:(core + 1) * sr])}
        m.update(consts)
        in_maps.append(m)
    res = run_bass_kernel_spmd(nc, in_maps, core_ids=list(range(N_CORES)),
                               trace=TRACE)
    last_exec_time_ns = res.exec_time_ns
    y = np.empty((B, C, H, W), np.float32)
    for core in range(N_CORES):
        b, s = core // 4, core % 4
        yc = np.asarray(res.results[core]["y"], dtype=np.float32)
        y[b, :, s * ROWS:(s + 1) * ROWS, :] = yc.reshape(C, ROWS, W)
    return y
